# revision 71
# baseline (speedup 1.0000x reference)
"""BiLevelRoutingAttention Trainium2 kernel.

TimelineSim device estimate ~246us/core vs ~546us for the v1 baseline.

Sharding: data-parallel over (T*B)=8 cores; core = b*4 + t.
Host: windowize + transpose + region-routing top-k (0.005% of FLOPs).
Device, per core (8192 tokens, 64 windows of 128):
  stage 1 (PE-bound, exact fp32 — spike bits flip for <1e-6
    perturbations near threshold): k,v token-major with the x-tile
    stationary; q computed directly TRANSPOSED (chan-major, Wq
    stationary) so no PE transposes are needed anywhere.
  stage 2 (DVE-bound) per window: routed kv as 8 half-width (N=129)
    bf16 matmuls accumulated over the topk windows (ones column ->
    ksum); masked block-diag kv + ksum-broadcast matrix (2 strided DVE
    ops) feed a transposed numerator matmul producing [attn^T |
    D-replicated] in one PSUM bank; eps-add on the scalar engine, fast
    approx reciprocal + scale on DVE; output projection straight from
    attn^T (bf16), f16 output DMA alternating both HWDGE queues.
  Stage 2 windows are INTERLEAVED into stage 1 as soon as their routed
  source windows are done, overlapping stage-2 DVE work under stage-1
  PE work.
The top-k indices (depend only on batch b) are baked into the program;
cores select their variant via tc.If(partition_id).
"""

import os
import numpy as np

# problem constants (hardcoded per contract)
T, B, Lt, Lh, Lw, C = 4, 2, 8, 32, 32, 256
WT, WH, WW = 4, 4, 4
NW = WT * WH * WW              # 64 windows
PT, PH, PW = Lt // WT, Lh // WH, Lw // WW
WS = PT * PH * PW              # 128 tokens per window
H, HD = 8, C // 8
TOPK = 4
NTOK = NW * WS                 # 8192 tokens per (t,b) shard
N_CORES = 8
NGRP = NW // 4                 # stage-1 token groups of 512

last_results = None            # stashed BassKernelResults for test harness
last_nc = None
last_in_maps = None


def _windowize(x):
    xw = x.reshape(T, B, WT, PT, WH, PH, WW, PW, C)
    xw = xw.transpose(0, 1, 2, 4, 6, 3, 5, 7, 8).reshape(T, B, NW, WS, C)
    return xw


def _unwindowize(ow):
    o = ow.reshape(T, B, WT, WH, WW, PT, PH, PW, C)
    o = o.transpose(0, 1, 2, 5, 3, 6, 4, 7, 8).reshape(T, B, Lt, Lh, Lw, C)
    return o


def _routing_idx(xw32):
    """Mimic reference routing in fp32: region scores -> top-4 window idx."""
    region = xw32.sum(0).mean(2)                           # [B,NW,C]
    scores = np.einsum('bic,bjc->bij', region, region) * np.float32(HD ** -0.5)
    # jax.lax.top_k tie-break = lowest index first; stable argsort matches
    idx = np.argsort(-scores, axis=-1, kind='stable')[:, :, :TOPK]
    return idx                                             # [B,NW,TOPK]


def _greedy_group_order(idx):
    """Order stage-1 groups so stage-2 windows unlock early."""
    need = [{int(j) // 4 for j in idx[n]} | {n // 4} for n in range(NW)]

    def rpos_of(order):
        pos = {g: p for p, g in enumerate(order)}
        return [max(pos[g] for g in need[n]) for n in range(NW)]

    def score_of(order):
        rpos = rpos_of(order)
        ready = [0] * NGRP
        for r in rpos:
            ready[r] += 1
        avail = 0
        for p in range(NGRP - 1):
            avail += ready[p]
            avail -= min(5, avail)
        tail = avail + ready[NGRP - 1]
        earliness = sum(NGRP - 1 - r for r in rpos)
        return (-tail, earliness)

    placed, order = set(), []
    while len(order) < NGRP:
        best, bestscore = None, None
        for g in range(NGRP):
            if g in placed:
                continue
            p2 = placed | {g}
            unlocked = sum(1 for nd in need if nd <= p2)
            partial = sum(len(nd & p2) / len(nd) for nd in need)
            score = (unlocked, partial)
            if bestscore is None or score > bestscore:
                best, bestscore = g, score
        order.append(best)
        placed.add(best)
    # hill-climb: minimize tail-window count, then maximize earliness
    best_s = score_of(order)
    improved = True
    while improved:
        improved = False
        for a in range(NGRP):
            for bgi in range(a + 1, NGRP):
                order[a], order[bgi] = order[bgi], order[a]
                s = score_of(order)
                if s > best_s:
                    best_s = s
                    improved = True
                else:
                    order[a], order[bgi] = order[bgi], order[a]
    return order, rpos_of(order)


def _build_program(idx_by_b, single_branch=False, repeat=1, bp_zero=False,
                   bq_zero=False):
    import concourse.bass as bass
    import concourse.mybir as mybir
    import concourse.tile as tile
    from concourse import bacc

    scheds = [_greedy_group_order(idx_by_b[0])]
    if not single_branch:
        scheds.append(_greedy_group_order(idx_by_b[1]))
    first_gs = [s[0][0] for s in scheds]

    f32 = mybir.dt.float32
    bf16 = mybir.dt.bfloat16
    f16 = mybir.dt.float16
    ge = mybir.AluOpType.is_ge
    mul = mybir.AluOpType.mult
    add = mybir.AluOpType.add

    nc = bacc.Bacc("TRN2", target_bir_lowering=False, debug=False,
                   num_devices=N_CORES)

    xwT = nc.dram_tensor("xwT", [C, NTOK], f32, kind="ExternalInput").ap()
    wq = nc.dram_tensor("wq", [C, 3 * C], f32, kind="ExternalInput").ap()
    bq = nc.dram_tensor("bq", [3 * C], f32, kind="ExternalInput").ap()
    wp = nc.dram_tensor("wp", [C, C], f32, kind="ExternalInput").ap()
    bp = nc.dram_tensor("bp", [C], f32, kind="ExternalInput").ap()
    masks = nc.dram_tensor("masks", [128, 128], f16, kind="ExternalInput").ap()
    out_d = nc.dram_tensor("out", [NTOK, C], f16, kind="ExternalOutput").ap()

    with tile.TileContext(nc) as tc:
        with (
            tc.tile_pool(name="const", bufs=1) as const_pool,
            tc.tile_pool(name="bits", bufs=1) as bits_pool,
            tc.tile_pool(name="work", bufs=4) as work_pool,
        ):
            # ---- resident constants (weights on SP queue first — the
            # first stage-1 matmul needs them; small consts on ACT) ----
            ones_row = const_pool.tile([1, 128], f32, tag="ones")
            nc.vector.memset(ones_row, 1.0)
            bq_row = const_pool.tile([1, 768], f32, tag="bqr")
            if not (bq_zero and bp_zero):
                nc.scalar.dma_start(bq_row, bq[None, :])
            eps_col = const_pool.tile([128, 1], f32, tag="eps")
            nc.vector.memset(eps_col, 1e-6)
            wkv_sb = const_pool.tile([128, 1024], f32, tag="wkv")
            nc.sync.dma_start(wkv_sb[:, 0:512], wq[0:128, 256:768])

            x_res = bits_pool.tile([128, 2 * NTOK], f32, tag="xres")

            def _x_piece(g, kc):
                eng = nc.sync if kc == 0 else nc.scalar
                eng.dma_start(
                    x_res[:, kc * NTOK + g * 512:kc * NTOK + (g + 1) * 512],
                    xwT[kc * 128:(kc + 1) * 128, g * 512:(g + 1) * 512])

            # first stage-1 group's x pieces beat the remaining const DMAs
            first_groups = []
            for gg in first_gs:
                if gg not in first_groups:
                    first_groups.append(gg)
            # first group in two sub-pieces: the very first window's
            # tokens land ahead of the rest
            def _x_subpiece(g, kc, c0, c1):
                eng = nc.sync if kc == 0 else nc.scalar
                eng.dma_start(
                    x_res[:, kc * NTOK + g * 512 + c0:
                          kc * NTOK + g * 512 + c1],
                    xwT[kc * 128:(kc + 1) * 128,
                        g * 512 + c0:g * 512 + c1])
            for g in first_groups:
                _x_subpiece(g, 0, 0, 128)
                _x_subpiece(g, 1, 0, 128)
                _x_subpiece(g, 0, 128, 512)
                _x_subpiece(g, 1, 128, 512)
            nc.sync.dma_start(wkv_sb[:, 512:1024], wq[128:256, 256:768])

            bp_row = const_pool.tile([1, 256], f32, tag="bpr")
            if not (bq_zero and bp_zero):
                nc.scalar.dma_start(bp_row, bp[None, :])
            mask_sb = const_pool.tile([128, 128], f16, tag="masks")
            nc.scalar.dma_start(mask_sb, masks)
            wqq_sb = const_pool.tile([128, 512], f32, tag="wqq")
            for kc in range(2):
                nc.scalar.dma_start(wqq_sb[:, kc * 256:(kc + 1) * 256],
                                    wq[kc * 128:(kc + 1) * 128, 0:256])
            wp32_sb = const_pool.tile([128, 512], f32, tag="wp32")
            for kc in range(2):
                nc.scalar.dma_start(wp32_sb[:, kc * 256:(kc + 1) * 256],
                                    wp[kc * 128:(kc + 1) * 128, :])

            wpb_sb = const_pool.tile([128, 512], bf16, tag="wpb")
            nc.vector.tensor_copy(wpb_sb, wp32_sb)

            thr_kv = const_pool.tile([128, 512], f32, tag="thrkv")
            thrq_neg = const_pool.tile([128, 2], f32, tag="thrqn")
            bp_bc = const_pool.tile([128, 256], f32, tag="bpbc")

            # ---- bit tensors (resident) ----
            k_bits = bits_pool.tile([128, NW * 256], bf16, tag="kb")
            v_ext = bits_pool.tile([128, NW * 258], bf16, tag="vb")
            v_r = v_ext.rearrange("p (w x) -> p w x", x=258)
            nc.vector.memset(v_r[:, :, 128], 1.0)
            nc.vector.memset(v_r[:, :, 257], 1.0)
            qt0 = bits_pool.tile([128, NTOK], f16, tag="qt0")
            qt1 = bits_pool.tile([128, NTOK], f16, tag="qt1")
            qt = (qt0, qt1)

            # ---- init: spike thresholds (spike(x+b) fires iff matmul
            # >= 2-b); plain memsets when the biases are all-zero ----
            if bq_zero and bp_zero:
                nc.vector.memset(thr_kv, 2.0)
                nc.vector.memset(thrq_neg, -2.0)
            else:
                with tc.tile_pool(name="init_ps", bufs=1,
                                  space="PSUM") as ips:
                    bc = ips.tile([128, 512], f32, tag="i0")
                    nc.tensor.matmul(bc, ones_row, bq_row[:, 256:768],
                                     start=True, stop=True)
                    nc.vector.tensor_scalar(out=thr_kv, in0=bc,
                                            scalar1=-1.0, scalar2=2.0,
                                            op0=mul, op1=add)
                    bc2 = ips.tile([128, 512], f32, tag="i1")
                    nc.tensor.matmul(bc2[:, 0:256], ones_row, bp_row,
                                     start=True, stop=True)
                    nc.vector.tensor_copy(bp_bc, bc2[:, 0:256])
                    bc3 = ips.tile([128, 512], f32, tag="i2")
                    for qd in range(2):
                        nc.tensor.matmul(bc3[:, qd:qd + 1],
                                         bq_row[:, qd * 128:(qd + 1) * 128],
                                         ones_row[:, 0:1], start=(qd == 0),
                                         stop=(qd == 1))
                    # qT spike via ACT: relu(sign(qp + (bq-2))) — bias per
                    # partition (= per q-channel in transposed layout)
                    nc.vector.tensor_scalar_add(out=thrq_neg,
                                                in0=bc3[:, 0:2],
                                                scalar1=-2.0)



            # ---- fused stage1 + interleaved stage2 ----
            def stage1_group(g, s1kv, s1q):
                xc0 = x_res[:, g * 512:(g + 1) * 512]
                xc1 = x_res[:, NTOK + g * 512:NTOK + (g + 1) * 512]
                for i in range(4):
                    w = 4 * g + i
                    ps = s1kv.tile([128, 512], f32, tag="kv")
                    nc.tensor.matmul(ps, xc0[:, i * 128:(i + 1) * 128],
                                     wkv_sb[:, 0:512],
                                     start=True, stop=False)
                    nc.tensor.matmul(
                        ps, xc1[:, i * 128:(i + 1) * 128],
                        wkv_sb[:, 512:1024], start=False, stop=True)
                    nc.vector.tensor_tensor(
                        out=k_bits[:, w * 256:(w + 1) * 256],
                        in0=ps[:, 0:256], in1=thr_kv[:, 0:256], op=ge)
                    # v halves -> (cols 0:128, 129:257) in one strided op
                    vv = v_r[:, w, 0:258].rearrange(
                        "p (t x) -> p t x", x=129)[:, :, 0:128]
                    nc.vector.tensor_tensor(
                        out=vv,
                        in0=ps[:, 256:512].rearrange(
                            "p (t x) -> p t x", x=128),
                        in1=thr_kv[:, 256:512].rearrange(
                            "p (t x) -> p t x", x=128),
                        op=ge)
                for qd in range(2):
                    qp = s1q.tile([128, 512], f32, tag="qt")
                    nc.tensor.matmul(qp,
                                     wqq_sb[:, qd * 128:(qd + 1) * 128],
                                     xc0, start=True, stop=False)
                    nc.tensor.matmul(
                        qp, wqq_sb[:, 256 + qd * 128:256 + (qd + 1) * 128],
                        xc1, start=False, stop=True)
                    sg = work_pool.tile([128, 512], f32, tag="sg")
                    nc.scalar.activation(
                        sg, qp, mybir.ActivationFunctionType.Sign,
                        bias=thrq_neg[:, qd:qd + 1])
                    nc.scalar.activation(
                        qt[qd][:, g * 512:(g + 1) * 512], sg,
                        mybir.ActivationFunctionType.Relu)

            def stage2_window(n, wi, idx, kv2, nmp, pjp, tail=False):
                js = [int(j) for j in idx[n]]
                kvp = kv2.tile([128, 512], f32, tag="kv")
                for jj, j in enumerate(js):
                    nc.tensor.matmul(
                        kvp[:, 0:129],
                        k_bits[:, j * 256:j * 256 + 128],
                        v_r[:, j, 0:129],
                        start=(jj == 0), stop=False)
                    nc.tensor.matmul(
                        kvp[:, 129:258],
                        k_bits[:, j * 256 + 128:(j + 1) * 256],
                        v_r[:, j, 129:258],
                        start=False, stop=(jj == 3))
                # masked block-diag kv + ksum-broadcast matrix (bf16, exact:
                # counts << 256); two strided DVE ops
                # evict kv to SBUF f16 on ACT (frees the PSUM bank early,
                # lets the kvJ DVE ops run all-2-byte at 2x rate)
                kvs = work_pool.tile([128, 258], f16, tag="kvs")
                nc.scalar.copy(kvs, kvp[:, 0:258])
                kvJ = work_pool.tile([128, 512], f16, tag="kvJ")
                kvv = kvs.rearrange("p (t x) -> p t x", x=129)
                mask_b = mask_sb.unsqueeze(1).to_broadcast([128, 2, 128])
                nc.vector.tensor_tensor(
                    out=kvJ[:, 0:256].rearrange("p (t x) -> p t x", x=128),
                    in0=kvv[:, :, 0:128], in1=mask_b, op=mul)
                nc.vector.tensor_tensor(
                    out=kvJ[:, 256:512].rearrange("p (t x) -> p t x", x=128),
                    in0=mask_b,
                    in1=kvv[:, :, 128:129].to_broadcast([128, 2, 128]),
                    op=mul)
                # transposed numerator + replicated denominator: one PSUM
                # bank, one accumulation group, 4 quarters
                nump = nmp.tile([128, 512], f32, tag="num")
                qs0 = qt0[:, n * 128:(n + 1) * 128]
                qs1 = qt1[:, n * 128:(n + 1) * 128]
                nc.tensor.matmul(nump[:, 0:128], kvJ[:, 0:128],
                                 qs0, start=True, stop=False)
                nc.tensor.matmul(nump[:, 128:256], kvJ[:, 128:256],
                                 qs1, start=False, stop=False)
                nc.tensor.matmul(nump[:, 256:384], kvJ[:, 256:384],
                                 qs0, start=False, stop=False)
                nc.tensor.matmul(nump[:, 384:512], kvJ[:, 384:512],
                                 qs1, start=False, stop=True)
                # attn = num / (D + 1e-6): eps-add+evict on ACT (idle
                # engine), fast approx reciprocal + scale on DVE
                deps = work_pool.tile([128, 256], f32, tag="deps")
                nc.scalar.add(deps, nump[:, 256:512], eps_col)
                rec = work_pool.tile([128, 256], f32, tag="rec")
                nc.vector.reciprocal_approx_fast(out=rec, in_=deps)
                attnT = work_pool.tile([128, 256], bf16, tag="attnT")
                nc.vector.tensor_tensor(
                    out=attnT, in0=nump[:, 0:256], in1=rec, op=mul)
                # output projection from attn^T (bf16)
                pj = pjp.tile([128, 512], f32, tag="pj")
                nc.tensor.matmul(pj[:, 0:256], attnT[:, 0:128],
                                 wpb_sb[:, 0:256], start=True, stop=False)
                nc.tensor.matmul(pj[:, 0:256], attnT[:, 128:256],
                                 wpb_sb[:, 256:512], start=False, stop=True)
                ob = work_pool.tile([128, 256], f16, tag="ob")
                if bp_zero:
                    nc.scalar.copy(ob, pj[:, 0:256])
                else:
                    nc.vector.tensor_tensor(out=ob, in0=pj[:, 0:256],
                                            in1=bp_bc, op=add)
                # tail: keep triggers off the busy ACT queue
                eng = nc.sync if (tail or wi % 2 == 0) else nc.scalar
                eng.dma_start(out_d[n * 128:(n + 1) * 128, :], ob)

            def body(idx, sched):
                gorder, rpos = sched
                # x pieces stream just-in-time with 2-group lookahead so
                # stage-1 group g never waits behind unrelated DMAs
                emitted = set(first_groups)

                def prefetch_x(upto):
                    for pp in range(min(upto + 1, NGRP)):
                        g = gorder[pp]
                        if g not in emitted:
                            emitted.add(g)
                            _x_piece(g, 0)
                            _x_piece(g, 1)

                ready = [[] for _ in range(NGRP)]
                for n in range(NW):
                    ready[rpos[n]].append(n)
                wi = 0
                # phase 1: stage-2 windows interleaved under stage-1 PE
                # shadow, <=3 per group (all DVE work must hide); shallow
                # stage-2 PSUM pools (8 banks total incl. stage-1 pools)
                avail = []
                tail = []
                with (
                    tc.tile_pool(name="s1kv_ps", bufs=2,
                                 space="PSUM") as s1kv,
                    tc.tile_pool(name="s1q_ps", bufs=2,
                                 space="PSUM") as s1q,
                ):
                    with (
                        tc.tile_pool(name="kv2a_ps", bufs=1,
                                     space="PSUM") as kv2,
                        tc.tile_pool(name="numa_ps", bufs=2,
                                     space="PSUM") as nmp,
                        tc.tile_pool(name="pja_ps", bufs=1,
                                     space="PSUM") as pjp,
                    ):
                        for p, g in enumerate(gorder):
                            prefetch_x(p + 2)
                            stage1_group(g, s1kv, s1q)
                            avail.extend(ready[p])
                            if p < NGRP - 1:
                                burst, avail = avail[:5], avail[5:]
                                for n in burst:
                                    stage2_window(n, wi, idx, kv2, nmp, pjp)
                                    wi += 1
                        tail = avail
                # phase 2: drain remaining windows with deep pools (the
                # stage-1 banks are free now)
                with (
                    tc.tile_pool(name="kv2b_ps", bufs=3,
                                 space="PSUM") as kv2,
                    tc.tile_pool(name="numb_ps", bufs=3,
                                 space="PSUM") as nmp,
                    tc.tile_pool(name="pjb_ps", bufs=2,
                                 space="PSUM") as pjp,
                ):
                    for n in tail:
                        stage2_window(n, wi, idx, kv2, nmp, pjp, tail=True)
                        wi += 1

            pid = None if single_branch else nc.partition_id()
            for _rep in range(repeat):
                if single_branch:
                    body(idx_by_b[0], scheds[0])
                else:
                    with tc.If(pid <= 3) as cmp:
                        body(idx_by_b[0], scheds[0])
                    with cmp.Else():
                        body(idx_by_b[1], scheds[1])

    nc.compile()
    return nc


def kernel(x, W_qkv, b_qkv, W_proj, b_proj):
    global last_results, last_nc, last_in_maps
    from concourse import bass_utils

    x = np.asarray(x, dtype=np.float32)
    xw = _windowize(x)                                     # [T,B,NW,WS,C]
    idx = _routing_idx(xw)                                 # [B,NW,TOPK]

    nc = _build_program(
        idx, bp_zero=bool(np.all(np.asarray(b_proj) == 0.0)),
        bq_zero=bool(np.all(np.asarray(b_qkv) == 0.0)))

    # same-head block mask: mask[d, e] = (d//32 == e//32)
    r = np.arange(128) // 32
    mask = (r[:, None] == r[None, :]).astype(np.float16)

    in_maps = []
    for core in range(N_CORES):
        b, t = divmod(core, T)
        xwT_c = np.ascontiguousarray(
            xw[t, b].reshape(NTOK, C).T)                   # [C, NTOK]
        in_maps.append({
            "xwT": xwT_c,
            "masks": mask,
            "wq": np.asarray(W_qkv, np.float32),
            "bq": np.asarray(b_qkv, np.float32),
            "wp": np.asarray(W_proj, np.float32),
            "bp": np.asarray(b_proj, np.float32),
        })

    res = bass_utils.run_bass_kernel_spmd(
        nc, in_maps, core_ids=list(range(N_CORES)), trace=False)
    last_results = res
    last_nc, last_in_maps = nc, in_maps

    ow = np.empty((T, B, NW, WS, C), np.float32)
    for core in range(N_CORES):
        b, t = divmod(core, T)
        ow[t, b] = res.results[core]["out"].astype(np.float32).reshape(
            NW, WS, C)
    return _unwindowize(ow)


# revision 73
# speedup vs baseline: 1.0070x; 1.0070x over previous
"""BiLevelRoutingAttention Trainium2 kernel.

TimelineSim device estimate ~246us/core vs ~546us for the v1 baseline.

Sharding: data-parallel over (T*B)=8 cores; core = b*4 + t.
Host: windowize + transpose + region-routing top-k (0.005% of FLOPs).
Device, per core (8192 tokens, 64 windows of 128):
  stage 1 (PE-bound, exact fp32 — spike bits flip for <1e-6
    perturbations near threshold): k,v token-major with the x-tile
    stationary; q computed directly TRANSPOSED (chan-major, Wq
    stationary) so no PE transposes are needed anywhere.
  stage 2 (DVE-bound) per window: routed kv as 8 half-width (N=129)
    bf16 matmuls accumulated over the topk windows (ones column ->
    ksum); masked block-diag kv + ksum-broadcast matrix (2 strided DVE
    ops) feed a transposed numerator matmul producing [attn^T |
    D-replicated] in one PSUM bank; eps-add on the scalar engine, fast
    approx reciprocal + scale on DVE; output projection straight from
    attn^T (bf16), f16 output DMA alternating both HWDGE queues.
  Stage 2 windows are INTERLEAVED into stage 1 as soon as their routed
  source windows are done, overlapping stage-2 DVE work under stage-1
  PE work.
The top-k indices (depend only on batch b) are baked into the program;
cores select their variant via tc.If(partition_id).
"""

import os
import numpy as np

# problem constants (hardcoded per contract)
T, B, Lt, Lh, Lw, C = 4, 2, 8, 32, 32, 256
WT, WH, WW = 4, 4, 4
NW = WT * WH * WW              # 64 windows
PT, PH, PW = Lt // WT, Lh // WH, Lw // WW
WS = PT * PH * PW              # 128 tokens per window
H, HD = 8, C // 8
TOPK = 4
NTOK = NW * WS                 # 8192 tokens per (t,b) shard
N_CORES = 8
NGRP = NW // 4                 # stage-1 token groups of 512

last_results = None            # stashed BassKernelResults for test harness
last_nc = None
last_in_maps = None


def _windowize(x):
    xw = x.reshape(T, B, WT, PT, WH, PH, WW, PW, C)
    xw = xw.transpose(0, 1, 2, 4, 6, 3, 5, 7, 8).reshape(T, B, NW, WS, C)
    return xw


def _unwindowize(ow):
    o = ow.reshape(T, B, WT, WH, WW, PT, PH, PW, C)
    o = o.transpose(0, 1, 2, 5, 3, 6, 4, 7, 8).reshape(T, B, Lt, Lh, Lw, C)
    return o


def _routing_idx(xw32):
    """Mimic reference routing in fp32: region scores -> top-4 window idx."""
    region = xw32.sum(0).mean(2)                           # [B,NW,C]
    scores = np.einsum('bic,bjc->bij', region, region) * np.float32(HD ** -0.5)
    # jax.lax.top_k tie-break = lowest index first; stable argsort matches
    idx = np.argsort(-scores, axis=-1, kind='stable')[:, :, :TOPK]
    return idx                                             # [B,NW,TOPK]


def _greedy_group_order(idx):
    """Order stage-1 groups so stage-2 windows unlock early."""
    need = [{int(j) // 4 for j in idx[n]} | {n // 4} for n in range(NW)]

    def rpos_of(order):
        pos = {g: p for p, g in enumerate(order)}
        return [max(pos[g] for g in need[n]) for n in range(NW)]

    def score_of(order):
        rpos = rpos_of(order)
        ready = [0] * NGRP
        for r in rpos:
            ready[r] += 1
        avail = 0
        for p in range(NGRP - 1):
            avail += ready[p]
            avail -= min(5, avail)
        tail = avail + ready[NGRP - 1]
        earliness = sum(NGRP - 1 - r for r in rpos)
        return (-tail, earliness)

    placed, order = set(), []
    while len(order) < NGRP:
        best, bestscore = None, None
        for g in range(NGRP):
            if g in placed:
                continue
            p2 = placed | {g}
            unlocked = sum(1 for nd in need if nd <= p2)
            partial = sum(len(nd & p2) / len(nd) for nd in need)
            score = (unlocked, partial)
            if bestscore is None or score > bestscore:
                best, bestscore = g, score
        order.append(best)
        placed.add(best)
    # hill-climb: minimize tail-window count, then maximize earliness
    best_s = score_of(order)
    improved = True
    while improved:
        improved = False
        for a in range(NGRP):
            for bgi in range(a + 1, NGRP):
                order[a], order[bgi] = order[bgi], order[a]
                s = score_of(order)
                if s > best_s:
                    best_s = s
                    improved = True
                else:
                    order[a], order[bgi] = order[bgi], order[a]
    return order, rpos_of(order)


def _build_program(idx_by_b, single_branch=False, repeat=1, bp_zero=False,
                   bq_zero=False):
    import concourse.bass as bass
    import concourse.mybir as mybir
    import concourse.tile as tile
    from concourse import bacc

    scheds = [_greedy_group_order(idx_by_b[0])]
    if not single_branch:
        scheds.append(_greedy_group_order(idx_by_b[1]))
    first_gs = [s[0][0] for s in scheds]

    f32 = mybir.dt.float32
    bf16 = mybir.dt.bfloat16
    f16 = mybir.dt.float16
    ge = mybir.AluOpType.is_ge
    mul = mybir.AluOpType.mult
    add = mybir.AluOpType.add

    nc = bacc.Bacc("TRN2", target_bir_lowering=False, debug=False,
                   num_devices=N_CORES)

    xwT = nc.dram_tensor("xwT", [C, NTOK], f32, kind="ExternalInput").ap()
    wq = nc.dram_tensor("wq", [C, 3 * C], f32, kind="ExternalInput").ap()
    bq = nc.dram_tensor("bq", [3 * C], f32, kind="ExternalInput").ap()
    wp = nc.dram_tensor("wp", [C, C], f32, kind="ExternalInput").ap()
    bp = nc.dram_tensor("bp", [C], f32, kind="ExternalInput").ap()
    masks = nc.dram_tensor("masks", [128, 128], f16, kind="ExternalInput").ap()
    out_d = nc.dram_tensor("out", [NTOK, C], f16, kind="ExternalOutput").ap()

    with tile.TileContext(nc) as tc:
        with (
            tc.tile_pool(name="const", bufs=1) as const_pool,
            tc.tile_pool(name="bits", bufs=1) as bits_pool,
            tc.tile_pool(name="work", bufs=4) as work_pool,
        ):
            # ---- resident constants (weights on SP queue first — the
            # first stage-1 matmul needs them; small consts on ACT) ----
            ones_row = const_pool.tile([1, 128], f32, tag="ones")
            nc.vector.memset(ones_row, 1.0)
            bq_row = const_pool.tile([1, 768], f32, tag="bqr")
            if not (bq_zero and bp_zero):
                nc.scalar.dma_start(bq_row, bq[None, :])
            eps_col = const_pool.tile([128, 1], f32, tag="eps")
            nc.vector.memset(eps_col, 1e-6)
            wkv_sb = const_pool.tile([128, 1024], f32, tag="wkv")
            nc.sync.dma_start(wkv_sb[:, 0:512], wq[0:128, 256:768])

            x_res = bits_pool.tile([128, 2 * NTOK], f32, tag="xres")

            def _x_piece(g, kc):
                eng = nc.sync if kc == 0 else nc.scalar
                eng.dma_start(
                    x_res[:, kc * NTOK + g * 512:kc * NTOK + (g + 1) * 512],
                    xwT[kc * 128:(kc + 1) * 128, g * 512:(g + 1) * 512])

            # first stage-1 group's x pieces beat the remaining const DMAs
            first_groups = []
            for gg in first_gs:
                if gg not in first_groups:
                    first_groups.append(gg)
            # first group in two sub-pieces: the very first window's
            # tokens land ahead of the rest
            def _x_subpiece(g, kc, c0, c1):
                eng = nc.sync if kc == 0 else nc.scalar
                eng.dma_start(
                    x_res[:, kc * NTOK + g * 512 + c0:
                          kc * NTOK + g * 512 + c1],
                    xwT[kc * 128:(kc + 1) * 128,
                        g * 512 + c0:g * 512 + c1])
            for g in first_groups:
                _x_subpiece(g, 0, 0, 128)
                _x_subpiece(g, 1, 0, 128)
                _x_subpiece(g, 0, 128, 512)
                _x_subpiece(g, 1, 128, 512)
            nc.sync.dma_start(wkv_sb[:, 512:1024], wq[128:256, 256:768])

            bp_row = const_pool.tile([1, 256], f32, tag="bpr")
            if not (bq_zero and bp_zero):
                nc.scalar.dma_start(bp_row, bp[None, :])
            mask_sb = const_pool.tile([128, 128], f16, tag="masks")
            nc.scalar.dma_start(mask_sb, masks)
            wqq_sb = const_pool.tile([128, 512], f32, tag="wqq")
            for kc in range(2):
                nc.scalar.dma_start(wqq_sb[:, kc * 256:(kc + 1) * 256],
                                    wq[kc * 128:(kc + 1) * 128, 0:256])
            wp32_sb = const_pool.tile([128, 512], f32, tag="wp32")
            for kc in range(2):
                nc.scalar.dma_start(wp32_sb[:, kc * 256:(kc + 1) * 256],
                                    wp[kc * 128:(kc + 1) * 128, :])

            wpb_sb = const_pool.tile([128, 512], bf16, tag="wpb")
            nc.vector.tensor_copy(wpb_sb, wp32_sb)

            thr_kv = const_pool.tile([128, 512], f32, tag="thrkv")
            thrq_neg = const_pool.tile([128, 2], f32, tag="thrqn")
            bp_bc = const_pool.tile([128, 256], f32, tag="bpbc")

            # ---- bit tensors (resident) ----
            k_bits = bits_pool.tile([128, NW * 256], bf16, tag="kb")
            v_ext = bits_pool.tile([128, NW * 258], bf16, tag="vb")
            v_r = v_ext.rearrange("p (w x) -> p w x", x=258)
            nc.vector.memset(v_r[:, :, 128], 1.0)
            nc.vector.memset(v_r[:, :, 257], 1.0)
            qt0 = bits_pool.tile([128, NTOK], f16, tag="qt0")
            qt1 = bits_pool.tile([128, NTOK], f16, tag="qt1")
            qt = (qt0, qt1)

            # ---- init: spike thresholds (spike(x+b) fires iff matmul
            # >= 2-b); plain memsets when the biases are all-zero ----
            if bq_zero and bp_zero:
                nc.vector.memset(thr_kv, 2.0)
                nc.vector.memset(thrq_neg, -2.0)
            else:
                with tc.tile_pool(name="init_ps", bufs=1,
                                  space="PSUM") as ips:
                    bc = ips.tile([128, 512], f32, tag="i0")
                    nc.tensor.matmul(bc, ones_row, bq_row[:, 256:768],
                                     start=True, stop=True)
                    nc.vector.tensor_scalar(out=thr_kv, in0=bc,
                                            scalar1=-1.0, scalar2=2.0,
                                            op0=mul, op1=add)
                    bc2 = ips.tile([128, 512], f32, tag="i1")
                    nc.tensor.matmul(bc2[:, 0:256], ones_row, bp_row,
                                     start=True, stop=True)
                    nc.vector.tensor_copy(bp_bc, bc2[:, 0:256])
                    bc3 = ips.tile([128, 512], f32, tag="i2")
                    for qd in range(2):
                        nc.tensor.matmul(bc3[:, qd:qd + 1],
                                         bq_row[:, qd * 128:(qd + 1) * 128],
                                         ones_row[:, 0:1], start=(qd == 0),
                                         stop=(qd == 1))
                    # qT spike via ACT: relu(sign(qp + (bq-2))) — bias per
                    # partition (= per q-channel in transposed layout)
                    nc.vector.tensor_scalar_add(out=thrq_neg,
                                                in0=bc3[:, 0:2],
                                                scalar1=-2.0)



            # ---- fused stage1 + interleaved stage2 ----
            def stage1_group(g, s1kv, s1q):
                xc0 = x_res[:, g * 512:(g + 1) * 512]
                xc1 = x_res[:, NTOK + g * 512:NTOK + (g + 1) * 512]
                for i in range(4):
                    w = 4 * g + i
                    ps = s1kv.tile([128, 512], f32, tag="kv")
                    nc.tensor.matmul(ps, xc0[:, i * 128:(i + 1) * 128],
                                     wkv_sb[:, 0:512],
                                     start=True, stop=False)
                    nc.tensor.matmul(
                        ps, xc1[:, i * 128:(i + 1) * 128],
                        wkv_sb[:, 512:1024], start=False, stop=True)
                    nc.vector.tensor_tensor(
                        out=k_bits[:, w * 256:(w + 1) * 256],
                        in0=ps[:, 0:256], in1=thr_kv[:, 0:256], op=ge)
                    # v halves -> (cols 0:128, 129:257) in one strided op
                    vv = v_r[:, w, 0:258].rearrange(
                        "p (t x) -> p t x", x=129)[:, :, 0:128]
                    nc.vector.tensor_tensor(
                        out=vv,
                        in0=ps[:, 256:512].rearrange(
                            "p (t x) -> p t x", x=128),
                        in1=thr_kv[:, 256:512].rearrange(
                            "p (t x) -> p t x", x=128),
                        op=ge)
                for qd in range(2):
                    qp = s1q.tile([128, 512], f32, tag="qt")
                    nc.tensor.matmul(qp,
                                     wqq_sb[:, qd * 128:(qd + 1) * 128],
                                     xc0, start=True, stop=False)
                    nc.tensor.matmul(
                        qp, wqq_sb[:, 256 + qd * 128:256 + (qd + 1) * 128],
                        xc1, start=False, stop=True)
                    sg = work_pool.tile([128, 512], f32, tag="sg")
                    nc.scalar.activation(
                        sg, qp, mybir.ActivationFunctionType.Sign,
                        bias=thrq_neg[:, qd:qd + 1])
                    nc.scalar.activation(
                        qt[qd][:, g * 512:(g + 1) * 512], sg,
                        mybir.ActivationFunctionType.Relu)

            def stage2_window(n, wi, idx, kv2, nmp, pjp, tail=False):
                js = [int(j) for j in idx[n]]
                kvp = kv2.tile([128, 512], f32, tag="kv")
                for jj, j in enumerate(js):
                    nc.tensor.matmul(
                        kvp[:, 0:129],
                        k_bits[:, j * 256:j * 256 + 128],
                        v_r[:, j, 0:129],
                        start=(jj == 0), stop=False)
                    nc.tensor.matmul(
                        kvp[:, 129:258],
                        k_bits[:, j * 256 + 128:(j + 1) * 256],
                        v_r[:, j, 129:258],
                        start=False, stop=(jj == 3))
                # masked block-diag kv + ksum-broadcast matrix (bf16, exact:
                # counts << 256); two strided DVE ops
                # evict kv to SBUF f16 on ACT (frees the PSUM bank early,
                # lets the kvJ DVE ops run all-2-byte at 2x rate)
                kvs = work_pool.tile([128, 258], f16, tag="kvs")
                nc.scalar.copy(kvs, kvp[:, 0:258])
                kvJ = work_pool.tile([128, 512], f16, tag="kvJ")
                kvv = kvs.rearrange("p (t x) -> p t x", x=129)
                mask_b = mask_sb.unsqueeze(1).to_broadcast([128, 2, 128])
                nc.vector.tensor_tensor(
                    out=kvJ[:, 0:256].rearrange("p (t x) -> p t x", x=128),
                    in0=kvv[:, :, 0:128], in1=mask_b, op=mul)
                nc.vector.tensor_tensor(
                    out=kvJ[:, 256:512].rearrange("p (t x) -> p t x", x=128),
                    in0=mask_b,
                    in1=kvv[:, :, 128:129].to_broadcast([128, 2, 128]),
                    op=mul)
                # transposed numerator + replicated denominator: one PSUM
                # bank, one accumulation group, 4 quarters
                nump = nmp.tile([128, 512], f32, tag="num")
                qs0 = qt0[:, n * 128:(n + 1) * 128]
                qs1 = qt1[:, n * 128:(n + 1) * 128]
                nc.tensor.matmul(nump[:, 0:128], kvJ[:, 0:128],
                                 qs0, start=True, stop=False)
                nc.tensor.matmul(nump[:, 128:256], kvJ[:, 128:256],
                                 qs1, start=False, stop=False)
                nc.tensor.matmul(nump[:, 256:384], kvJ[:, 256:384],
                                 qs0, start=False, stop=False)
                nc.tensor.matmul(nump[:, 384:512], kvJ[:, 384:512],
                                 qs1, start=False, stop=True)
                # attn = num / (D + 1e-6): eps-add+evict on ACT (idle
                # engine), fast approx reciprocal + scale on DVE
                deps = work_pool.tile([128, 256], f32, tag="deps")
                nc.scalar.add(deps, nump[:, 256:512], eps_col)
                rec = work_pool.tile([128, 256], f32, tag="rec")
                nc.vector.reciprocal_approx_fast(out=rec, in_=deps)
                attnT = work_pool.tile([128, 256], bf16, tag="attnT")
                nc.vector.tensor_tensor(
                    out=attnT, in0=nump[:, 0:256], in1=rec, op=mul)
                # output projection from attn^T (bf16)
                pj = pjp.tile([128, 512], f32, tag="pj")
                nc.tensor.matmul(pj[:, 0:256], attnT[:, 0:128],
                                 wpb_sb[:, 0:256], start=True, stop=False)
                nc.tensor.matmul(pj[:, 0:256], attnT[:, 128:256],
                                 wpb_sb[:, 256:512], start=False, stop=True)
                ob = work_pool.tile([128, 256], f16, tag="ob")
                if bp_zero:
                    nc.scalar.copy(ob, pj[:, 0:256])
                else:
                    nc.vector.tensor_tensor(out=ob, in0=pj[:, 0:256],
                                            in1=bp_bc, op=add)
                # tail: keep triggers off the busy ACT queue
                eng = nc.sync if (tail or wi % 2 == 0) else nc.scalar
                eng.dma_start(out_d[n * 128:(n + 1) * 128, :], ob)

            def body(idx, sched):
                gorder, rpos = sched
                # x pieces stream just-in-time with 2-group lookahead so
                # stage-1 group g never waits behind unrelated DMAs
                emitted = set(first_groups)

                def prefetch_x(upto):
                    for pp in range(min(upto + 1, NGRP)):
                        g = gorder[pp]
                        if g not in emitted:
                            emitted.add(g)
                            _x_piece(g, 0)
                            _x_piece(g, 1)

                ready = [[] for _ in range(NGRP)]
                for n in range(NW):
                    ready[rpos[n]].append(n)
                wi = 0
                # phase 1: stage-2 windows interleaved under stage-1 PE
                # shadow, <=3 per group (all DVE work must hide); shallow
                # stage-2 PSUM pools (8 banks total incl. stage-1 pools)
                avail = []
                tail = []
                with (
                    tc.tile_pool(name="s1kv_ps", bufs=2,
                                 space="PSUM") as s1kv,
                    tc.tile_pool(name="s1q_ps", bufs=2,
                                 space="PSUM") as s1q,
                ):
                    with (
                        tc.tile_pool(name="kv2a_ps", bufs=1,
                                     space="PSUM") as kv2,
                        tc.tile_pool(name="numa_ps", bufs=2,
                                     space="PSUM") as nmp,
                        tc.tile_pool(name="pja_ps", bufs=1,
                                     space="PSUM") as pjp,
                    ):
                        for p, g in enumerate(gorder):
                            prefetch_x(p + 2)
                            stage1_group(g, s1kv, s1q)
                            avail.extend(ready[p])
                            if p < NGRP - 1:
                                burst, avail = avail[:5], avail[5:]
                                for n in burst:
                                    stage2_window(n, wi, idx, kv2, nmp, pjp)
                                    wi += 1
                        tail = avail
                # phase 2: drain remaining windows with deep pools (the
                # stage-1 banks are free now)
                with (
                    tc.tile_pool(name="kv2b_ps", bufs=3,
                                 space="PSUM") as kv2,
                    tc.tile_pool(name="numb_ps", bufs=3,
                                 space="PSUM") as nmp,
                    tc.tile_pool(name="pjb_ps", bufs=2,
                                 space="PSUM") as pjp,
                ):
                    for n in tail:
                        stage2_window(n, wi, idx, kv2, nmp, pjp, tail=True)
                        wi += 1

            pid = None if single_branch else nc.partition_id()
            for _rep in range(repeat):
                if single_branch:
                    body(idx_by_b[0], scheds[0])
                else:
                    with tc.If(pid <= 3) as cmp:
                        body(idx_by_b[0], scheds[0])
                    with cmp.Else():
                        body(idx_by_b[1], scheds[1])

    nc.compile()
    return nc


def kernel(x, W_qkv, b_qkv, W_proj, b_proj):
    global last_results, last_nc, last_in_maps
    from concourse import bass_utils

    x = np.asarray(x, dtype=np.float32)
    xw = _windowize(x)                                     # [T,B,NW,WS,C]
    idx = _routing_idx(xw)                                 # [B,NW,TOPK]

    nc = _build_program(
        idx, bp_zero=bool(np.all(np.asarray(b_proj) == 0.0)),
        bq_zero=bool(np.all(np.asarray(b_qkv) == 0.0)))

    # same-head block mask: mask[d, e] = (d//32 == e//32)
    r = np.arange(128) // 32
    mask = (r[:, None] == r[None, :]).astype(np.float16)

    in_maps = []
    for core in range(N_CORES):
        b, t = divmod(core, T)
        xwT_c = np.ascontiguousarray(
            xw[t, b].reshape(NTOK, C).T)                   # [C, NTOK]
        in_maps.append({
            "xwT": xwT_c,
            "masks": mask,
            "wq": np.asarray(W_qkv, np.float32),
            "bq": np.asarray(b_qkv, np.float32),
            "wp": np.asarray(W_proj, np.float32),
            "bp": np.asarray(b_proj, np.float32),
        })

    res = bass_utils.run_bass_kernel_spmd(
        nc, in_maps, core_ids=list(range(N_CORES)), trace=False)
    last_results = res
    last_nc, last_in_maps = nc, in_maps

    ow = np.empty((T, B, NW, WS, C), np.float32)
    for core in range(N_CORES):
        b, t = divmod(core, T)
        ow[t, b] = res.results[core]["out"].astype(np.float32).reshape(
            NW, WS, C)
    return _unwindowize(ow)


# revision 75
# speedup vs baseline: 1.0401x; 1.0329x over previous
"""BiLevelRoutingAttention Trainium2 kernel.

TimelineSim device estimate ~246us/core vs ~546us for the v1 baseline.

Sharding: data-parallel over (T*B)=8 cores; core = b*4 + t.
Host: windowize + transpose + region-routing top-k (0.005% of FLOPs).
Device, per core (8192 tokens, 64 windows of 128):
  stage 1 (PE-bound, exact fp32 — spike bits flip for <1e-6
    perturbations near threshold): k,v token-major with the x-tile
    stationary; q computed directly TRANSPOSED (chan-major, Wq
    stationary) so no PE transposes are needed anywhere.
  stage 2 (DVE-bound) per window: routed kv as 8 half-width (N=129)
    bf16 matmuls accumulated over the topk windows (ones column ->
    ksum); masked block-diag kv + ksum-broadcast matrix (2 strided DVE
    ops) feed a transposed numerator matmul producing [attn^T |
    D-replicated] in one PSUM bank; eps-add on the scalar engine, fast
    approx reciprocal + scale on DVE; output projection straight from
    attn^T (bf16), f16 output DMA alternating both HWDGE queues.
  Stage 2 windows are INTERLEAVED into stage 1 as soon as their routed
  source windows are done, overlapping stage-2 DVE work under stage-1
  PE work.
The top-k indices (depend only on batch b) are baked into the program;
cores select their variant via tc.If(partition_id).
"""

import os
import numpy as np

# problem constants (hardcoded per contract)
T, B, Lt, Lh, Lw, C = 4, 2, 8, 32, 32, 256
WT, WH, WW = 4, 4, 4
NW = WT * WH * WW              # 64 windows
PT, PH, PW = Lt // WT, Lh // WH, Lw // WW
WS = PT * PH * PW              # 128 tokens per window
H, HD = 8, C // 8
TOPK = 4
NTOK = NW * WS                 # 8192 tokens per (t,b) shard
N_CORES = 8
NGRP = NW // 4                 # stage-1 token groups of 512

last_results = None            # stashed BassKernelResults for test harness
last_nc = None
last_in_maps = None


def _windowize(x):
    xw = x.reshape(T, B, WT, PT, WH, PH, WW, PW, C)
    xw = xw.transpose(0, 1, 2, 4, 6, 3, 5, 7, 8).reshape(T, B, NW, WS, C)
    return xw


def _unwindowize(ow):
    o = ow.reshape(T, B, WT, WH, WW, PT, PH, PW, C)
    o = o.transpose(0, 1, 2, 5, 3, 6, 4, 7, 8).reshape(T, B, Lt, Lh, Lw, C)
    return o


def _routing_idx(xw32):
    """Mimic reference routing in fp32: region scores -> top-4 window idx."""
    region = xw32.sum(0).mean(2)                           # [B,NW,C]
    scores = np.einsum('bic,bjc->bij', region, region) * np.float32(HD ** -0.5)
    # jax.lax.top_k tie-break = lowest index first; stable argsort matches
    idx = np.argsort(-scores, axis=-1, kind='stable')[:, :, :TOPK]
    return idx                                             # [B,NW,TOPK]


def _greedy_group_order(idx):
    """Order stage-1 groups so stage-2 windows unlock early."""
    need = [{int(j) // 4 for j in idx[n]} | {n // 4} for n in range(NW)]

    def rpos_of(order):
        pos = {g: p for p, g in enumerate(order)}
        return [max(pos[g] for g in need[n]) for n in range(NW)]

    def score_of(order):
        rpos = rpos_of(order)
        ready = [0] * NGRP
        for r in rpos:
            ready[r] += 1
        avail = 0
        for p in range(NGRP - 1):
            avail += ready[p]
            avail -= min(5, avail)
        tail = avail + ready[NGRP - 1]
        earliness = sum(NGRP - 1 - r for r in rpos)
        return (-tail, earliness)

    placed, order = set(), []
    while len(order) < NGRP:
        best, bestscore = None, None
        for g in range(NGRP):
            if g in placed:
                continue
            p2 = placed | {g}
            unlocked = sum(1 for nd in need if nd <= p2)
            partial = sum(len(nd & p2) / len(nd) for nd in need)
            score = (unlocked, partial)
            if bestscore is None or score > bestscore:
                best, bestscore = g, score
        order.append(best)
        placed.add(best)
    # hill-climb: minimize tail-window count, then maximize earliness
    best_s = score_of(order)
    improved = True
    while improved:
        improved = False
        for a in range(NGRP):
            for bgi in range(a + 1, NGRP):
                order[a], order[bgi] = order[bgi], order[a]
                s = score_of(order)
                if s > best_s:
                    best_s = s
                    improved = True
                else:
                    order[a], order[bgi] = order[bgi], order[a]
    return order, rpos_of(order)


def _build_program(idx_by_b, single_branch=False, repeat=1, bp_zero=False,
                   bq_zero=False):
    import concourse.bass as bass
    import concourse.mybir as mybir
    import concourse.tile as tile
    from concourse import bacc

    scheds = [_greedy_group_order(idx_by_b[0])]
    if not single_branch:
        scheds.append(_greedy_group_order(idx_by_b[1]))
    first_gs = [s[0][0] for s in scheds]

    f32 = mybir.dt.float32
    bf16 = mybir.dt.bfloat16
    f16 = mybir.dt.float16
    ge = mybir.AluOpType.is_ge
    mul = mybir.AluOpType.mult
    add = mybir.AluOpType.add

    nc = bacc.Bacc("TRN2", target_bir_lowering=False, debug=False,
                   num_devices=N_CORES)

    xwT = nc.dram_tensor("xwT", [C, NTOK], f32, kind="ExternalInput").ap()
    wq = nc.dram_tensor("wq", [C, 3 * C], f32, kind="ExternalInput").ap()
    bq = nc.dram_tensor("bq", [3 * C], f32, kind="ExternalInput").ap()
    wp = nc.dram_tensor("wp", [C, C], f32, kind="ExternalInput").ap()
    bp = nc.dram_tensor("bp", [C], f32, kind="ExternalInput").ap()
    masks = nc.dram_tensor("masks", [128, 128], f16, kind="ExternalInput").ap()
    out_d = nc.dram_tensor("out", [NTOK, C], f16, kind="ExternalOutput").ap()

    with tile.TileContext(nc) as tc:
        with (
            tc.tile_pool(name="const", bufs=1) as const_pool,
            tc.tile_pool(name="bits", bufs=1) as bits_pool,
            tc.tile_pool(name="work", bufs=4) as work_pool,
        ):
            # ---- resident constants (weights on SP queue first — the
            # first stage-1 matmul needs them; small consts on ACT) ----
            ones_row = const_pool.tile([1, 128], f32, tag="ones")
            nc.vector.memset(ones_row, 1.0)
            bq_row = const_pool.tile([1, 768], f32, tag="bqr")
            if not (bq_zero and bp_zero):
                nc.scalar.dma_start(bq_row, bq[None, :])
            eps_col = const_pool.tile([128, 1], f32, tag="eps")
            nc.vector.memset(eps_col, 1e-6)
            wkv_sb = const_pool.tile([128, 1024], f32, tag="wkv")
            nc.sync.dma_start(wkv_sb[:, 0:512], wq[0:128, 256:768])

            x_res = bits_pool.tile([128, 2 * NTOK], f32, tag="xres")

            def _x_piece(g, kc):
                eng = nc.sync if kc == 0 else nc.scalar
                eng.dma_start(
                    x_res[:, kc * NTOK + g * 512:kc * NTOK + (g + 1) * 512],
                    xwT[kc * 128:(kc + 1) * 128, g * 512:(g + 1) * 512])

            # first stage-1 group's x pieces beat the remaining const DMAs
            first_groups = []
            for gg in first_gs:
                if gg not in first_groups:
                    first_groups.append(gg)
            # first group in two sub-pieces: the very first window's
            # tokens land ahead of the rest
            def _x_subpiece(g, kc, c0, c1):
                eng = nc.sync if kc == 0 else nc.scalar
                eng.dma_start(
                    x_res[:, kc * NTOK + g * 512 + c0:
                          kc * NTOK + g * 512 + c1],
                    xwT[kc * 128:(kc + 1) * 128,
                        g * 512 + c0:g * 512 + c1])
            for g in first_groups:
                _x_subpiece(g, 0, 0, 128)
                _x_subpiece(g, 1, 0, 128)
                _x_subpiece(g, 0, 128, 512)
                _x_subpiece(g, 1, 128, 512)
            nc.sync.dma_start(wkv_sb[:, 512:1024], wq[128:256, 256:768])

            bp_row = const_pool.tile([1, 256], f32, tag="bpr")
            if not (bq_zero and bp_zero):
                nc.scalar.dma_start(bp_row, bp[None, :])
            mask_sb = const_pool.tile([128, 128], f16, tag="masks")
            nc.scalar.dma_start(mask_sb, masks)
            wqq_sb = const_pool.tile([128, 512], f32, tag="wqq")
            for kc in range(2):
                nc.scalar.dma_start(wqq_sb[:, kc * 256:(kc + 1) * 256],
                                    wq[kc * 128:(kc + 1) * 128, 0:256])
            wp32_sb = const_pool.tile([128, 512], f32, tag="wp32")
            for kc in range(2):
                nc.scalar.dma_start(wp32_sb[:, kc * 256:(kc + 1) * 256],
                                    wp[kc * 128:(kc + 1) * 128, :])

            wpb_sb = const_pool.tile([128, 512], bf16, tag="wpb")
            nc.vector.tensor_copy(wpb_sb, wp32_sb)

            thr_kv = const_pool.tile([128, 512], f32, tag="thrkv")
            thrq_neg = const_pool.tile([128, 2], f32, tag="thrqn")
            bp_bc = const_pool.tile([128, 256], f32, tag="bpbc")

            # ---- bit tensors (resident) ----
            k_bits = bits_pool.tile([128, NW * 256], bf16, tag="kb")
            v_ext = bits_pool.tile([128, NW * 258], bf16, tag="vb")
            v_r = v_ext.rearrange("p (w x) -> p w x", x=258)
            nc.vector.memset(v_r[:, :, 128], 1.0)
            nc.vector.memset(v_r[:, :, 257], 1.0)
            qt0 = bits_pool.tile([128, NTOK], f16, tag="qt0")
            qt1 = bits_pool.tile([128, NTOK], f16, tag="qt1")
            qt = (qt0, qt1)

            # ---- init: spike thresholds (spike(x+b) fires iff matmul
            # >= 2-b); plain memsets when the biases are all-zero ----
            if bq_zero and bp_zero:
                nc.vector.memset(thr_kv, 2.0)
                nc.vector.memset(thrq_neg, -2.0)
            else:
                with tc.tile_pool(name="init_ps", bufs=1,
                                  space="PSUM") as ips:
                    bc = ips.tile([128, 512], f32, tag="i0")
                    nc.tensor.matmul(bc, ones_row, bq_row[:, 256:768],
                                     start=True, stop=True)
                    nc.vector.tensor_scalar(out=thr_kv, in0=bc,
                                            scalar1=-1.0, scalar2=2.0,
                                            op0=mul, op1=add)
                    bc2 = ips.tile([128, 512], f32, tag="i1")
                    nc.tensor.matmul(bc2[:, 0:256], ones_row, bp_row,
                                     start=True, stop=True)
                    nc.vector.tensor_copy(bp_bc, bc2[:, 0:256])
                    bc3 = ips.tile([128, 512], f32, tag="i2")
                    for qd in range(2):
                        nc.tensor.matmul(bc3[:, qd:qd + 1],
                                         bq_row[:, qd * 128:(qd + 1) * 128],
                                         ones_row[:, 0:1], start=(qd == 0),
                                         stop=(qd == 1))
                    # qT spike via ACT: relu(sign(qp + (bq-2))) — bias per
                    # partition (= per q-channel in transposed layout)
                    nc.vector.tensor_scalar_add(out=thrq_neg,
                                                in0=bc3[:, 0:2],
                                                scalar1=-2.0)



            # ---- fused stage1 + interleaved stage2 ----
            def stage1_group(g, s1kv, s1q):
                xc0 = x_res[:, g * 512:(g + 1) * 512]
                xc1 = x_res[:, NTOK + g * 512:NTOK + (g + 1) * 512]
                for i in range(4):
                    w = 4 * g + i
                    ps = s1kv.tile([128, 512], f32, tag="kv")
                    nc.tensor.matmul(ps, xc0[:, i * 128:(i + 1) * 128],
                                     wkv_sb[:, 0:512],
                                     start=True, stop=False)
                    nc.tensor.matmul(
                        ps, xc1[:, i * 128:(i + 1) * 128],
                        wkv_sb[:, 512:1024], start=False, stop=True)
                    nc.vector.tensor_tensor(
                        out=k_bits[:, w * 256:(w + 1) * 256],
                        in0=ps[:, 0:256], in1=thr_kv[:, 0:256], op=ge)
                    # v halves -> (cols 0:128, 129:257) in one strided op
                    vv = v_r[:, w, 0:258].rearrange(
                        "p (t x) -> p t x", x=129)[:, :, 0:128]
                    nc.vector.tensor_tensor(
                        out=vv,
                        in0=ps[:, 256:512].rearrange(
                            "p (t x) -> p t x", x=128),
                        in1=thr_kv[:, 256:512].rearrange(
                            "p (t x) -> p t x", x=128),
                        op=ge)
                for qd in range(2):
                    qp = s1q.tile([128, 512], f32, tag="qt")
                    nc.tensor.matmul(qp,
                                     wqq_sb[:, qd * 128:(qd + 1) * 128],
                                     xc0, start=True, stop=False)
                    nc.tensor.matmul(
                        qp, wqq_sb[:, 256 + qd * 128:256 + (qd + 1) * 128],
                        xc1, start=False, stop=True)
                    sg = work_pool.tile([128, 512], f32, tag="sg")
                    nc.scalar.activation(
                        sg, qp, mybir.ActivationFunctionType.Sign,
                        bias=thrq_neg[:, qd:qd + 1])
                    nc.scalar.activation(
                        qt[qd][:, g * 512:(g + 1) * 512], sg,
                        mybir.ActivationFunctionType.Relu)

            def stage2_window(n, wi, idx, kv2, nmp, pjp, tail=False):
                js = [int(j) for j in idx[n]]
                kvp = kv2.tile([128, 512], f32, tag="kv")
                for jj, j in enumerate(js):
                    nc.tensor.matmul(
                        kvp[:, 0:129],
                        k_bits[:, j * 256:j * 256 + 128],
                        v_r[:, j, 0:129],
                        start=(jj == 0), stop=False)
                    nc.tensor.matmul(
                        kvp[:, 129:258],
                        k_bits[:, j * 256 + 128:(j + 1) * 256],
                        v_r[:, j, 129:258],
                        start=False, stop=(jj == 3))
                # masked block-diag kv + ksum-broadcast matrix (bf16, exact:
                # counts << 256); two strided DVE ops
                # evict kv to SBUF f16 on ACT (frees the PSUM bank early,
                # lets the kvJ DVE ops run all-2-byte at 2x rate)
                kvs = work_pool.tile([128, 258], f16, tag="kvs")
                nc.scalar.copy(kvs, kvp[:, 0:258])
                kvJ = work_pool.tile([128, 512], f16, tag="kvJ")
                kvv = kvs.rearrange("p (t x) -> p t x", x=129)
                mask_b = mask_sb.unsqueeze(1).to_broadcast([128, 2, 128])
                nc.vector.tensor_tensor(
                    out=kvJ[:, 0:256].rearrange("p (t x) -> p t x", x=128),
                    in0=kvv[:, :, 0:128], in1=mask_b, op=mul)
                nc.vector.tensor_tensor(
                    out=kvJ[:, 256:512].rearrange("p (t x) -> p t x", x=128),
                    in0=mask_b,
                    in1=kvv[:, :, 128:129].to_broadcast([128, 2, 128]),
                    op=mul)
                # transposed numerator + replicated denominator: one PSUM
                # bank, one accumulation group, 4 quarters
                nump = nmp.tile([128, 512], f32, tag="num")
                qs0 = qt0[:, n * 128:(n + 1) * 128]
                qs1 = qt1[:, n * 128:(n + 1) * 128]
                nc.tensor.matmul(nump[:, 0:128], kvJ[:, 0:128],
                                 qs0, start=True, stop=False)
                nc.tensor.matmul(nump[:, 128:256], kvJ[:, 128:256],
                                 qs1, start=False, stop=False)
                nc.tensor.matmul(nump[:, 256:384], kvJ[:, 256:384],
                                 qs0, start=False, stop=False)
                nc.tensor.matmul(nump[:, 384:512], kvJ[:, 384:512],
                                 qs1, start=False, stop=True)
                # attn = num / (D + 1e-6): eps-add+evict on ACT (idle
                # engine), fast approx reciprocal + scale on DVE
                deps = work_pool.tile([128, 256], f32, tag="deps")
                nc.scalar.add(deps, nump[:, 256:512], eps_col)
                rec = work_pool.tile([128, 256], f32, tag="rec")
                nc.vector.reciprocal_approx_fast(out=rec, in_=deps)
                attnT = work_pool.tile([128, 256], bf16, tag="attnT")
                nc.vector.tensor_tensor(
                    out=attnT, in0=nump[:, 0:256], in1=rec, op=mul)
                # output projection from attn^T (bf16)
                pj = pjp.tile([128, 512], f32, tag="pj")
                nc.tensor.matmul(pj[:, 0:256], attnT[:, 0:128],
                                 wpb_sb[:, 0:256], start=True, stop=False)
                nc.tensor.matmul(pj[:, 0:256], attnT[:, 128:256],
                                 wpb_sb[:, 256:512], start=False, stop=True)
                ob = work_pool.tile([128, 256], f16, tag="ob")
                if bp_zero:
                    nc.scalar.copy(ob, pj[:, 0:256])
                else:
                    nc.vector.tensor_tensor(out=ob, in0=pj[:, 0:256],
                                            in1=bp_bc, op=add)
                # tail: keep triggers off the busy ACT queue
                eng = nc.sync if (tail or wi % 2 == 0) else nc.scalar
                eng.dma_start(out_d[n * 128:(n + 1) * 128, :], ob)

            def body(idx, sched):
                gorder, rpos = sched
                # x pieces stream just-in-time with 2-group lookahead so
                # stage-1 group g never waits behind unrelated DMAs
                emitted = set(first_groups)

                def prefetch_x(upto):
                    for pp in range(min(upto + 1, NGRP)):
                        g = gorder[pp]
                        if g not in emitted:
                            emitted.add(g)
                            _x_piece(g, 0)
                            _x_piece(g, 1)

                ready = [[] for _ in range(NGRP)]
                for n in range(NW):
                    ready[rpos[n]].append(n)
                wi = 0
                # phase 1: stage-2 windows interleaved under stage-1 PE
                # shadow, <=3 per group (all DVE work must hide); shallow
                # stage-2 PSUM pools (8 banks total incl. stage-1 pools)
                avail = []
                tail = []
                with (
                    tc.tile_pool(name="s1kv_ps", bufs=2,
                                 space="PSUM") as s1kv,
                    tc.tile_pool(name="s1q_ps", bufs=2,
                                 space="PSUM") as s1q,
                ):
                    with (
                        tc.tile_pool(name="kv2a_ps", bufs=1,
                                     space="PSUM") as kv2,
                        tc.tile_pool(name="numa_ps", bufs=2,
                                     space="PSUM") as nmp,
                        tc.tile_pool(name="pja_ps", bufs=1,
                                     space="PSUM") as pjp,
                    ):
                        for p, g in enumerate(gorder):
                            prefetch_x(p + 2)
                            stage1_group(g, s1kv, s1q)
                            avail.extend(ready[p])
                            if p < NGRP - 1:
                                burst, avail = avail[:5], avail[5:]
                                for n in burst:
                                    stage2_window(n, wi, idx, kv2, nmp, pjp)
                                    wi += 1
                        tail = avail
                # phase 2: drain remaining windows with deep pools (the
                # stage-1 banks are free now)
                with (
                    tc.tile_pool(name="kv2b_ps", bufs=3,
                                 space="PSUM") as kv2,
                    tc.tile_pool(name="numb_ps", bufs=3,
                                 space="PSUM") as nmp,
                    tc.tile_pool(name="pjb_ps", bufs=2,
                                 space="PSUM") as pjp,
                ):
                    for n in tail:
                        stage2_window(n, wi, idx, kv2, nmp, pjp, tail=True)
                        wi += 1

            pid = None if single_branch else nc.partition_id()
            for _rep in range(repeat):
                if single_branch:
                    body(idx_by_b[0], scheds[0])
                else:
                    with tc.If(pid <= 3) as cmp:
                        body(idx_by_b[0], scheds[0])
                    with cmp.Else():
                        body(idx_by_b[1], scheds[1])

    nc.compile()
    return nc


def kernel(x, W_qkv, b_qkv, W_proj, b_proj):
    global last_results, last_nc, last_in_maps
    from concourse import bass_utils

    x = np.asarray(x, dtype=np.float32)
    xw = _windowize(x)                                     # [T,B,NW,WS,C]
    idx = _routing_idx(xw)                                 # [B,NW,TOPK]

    nc = _build_program(
        idx, bp_zero=bool(np.all(np.asarray(b_proj) == 0.0)),
        bq_zero=bool(np.all(np.asarray(b_qkv) == 0.0)))

    # same-head block mask: mask[d, e] = (d//32 == e//32)
    r = np.arange(128) // 32
    mask = (r[:, None] == r[None, :]).astype(np.float16)

    in_maps = []
    for core in range(N_CORES):
        b, t = divmod(core, T)
        xwT_c = np.ascontiguousarray(
            xw[t, b].reshape(NTOK, C).T)                   # [C, NTOK]
        in_maps.append({
            "xwT": xwT_c,
            "masks": mask,
            "wq": np.asarray(W_qkv, np.float32),
            "bq": np.asarray(b_qkv, np.float32),
            "wp": np.asarray(W_proj, np.float32),
            "bp": np.asarray(b_proj, np.float32),
        })

    res = bass_utils.run_bass_kernel_spmd(
        nc, in_maps, core_ids=list(range(N_CORES)), trace=False)
    last_results = res
    last_nc, last_in_maps = nc, in_maps

    ow = np.empty((T, B, NW, WS, C), np.float32)
    for core in range(N_CORES):
        b, t = divmod(core, T)
        ow[t, b] = res.results[core]["out"].astype(np.float32).reshape(
            NW, WS, C)
    return _unwindowize(ow)
